# revision 38
# baseline (speedup 1.0000x reference)
"""NerfHead Trainium2 kernel: per-sample generated 2-layer MLP over pixels.

Sharding: pure data parallel over the batch dim across 8 cores.
Host does all layout permutations / dtype casts (restaging only).

Per core (B=256 samples):
  Phase 1: params j-tiles [128, B] PSUM (pair-batched ACT evac when bias
    is all-zero) into big [128(i), B, NJ] so per-sample mlp stationaries
    are CONTIGUOUS (fast LDWEIGHTS). W streamed as 1.5MB slab-major
    host-prepped DMAs, alternating sync/scalar HWDGE queues; the first
    slab is split per k-chunk across both queues so matmuls start ~5us
    in (earlier HAM warm-up). cn2 via squares + masked-ones MM
    accumulation. Pixel RMS stats spread UNIFORMLY over the whole phase
    (~1.06 pairs/iter) on gpsimd SWDGE granule loads + granule-sized
    DVE squares, so the stats DMA never oversubscribes the W stream and
    the masked-ones ms matmuls never stall the PE queue.
  Normalizers: batched ACT Sqrt + DVE reciprocal.
  Phase 2 (per quad = 4 samples): 2 sbc broadcast MMs (ident-col,
    N=512) into a 2-bank PSUM quad -> ONE DVE stt xn = pp*nw*inv ->
    mlp1 -> ACT Silu(scale=inv_cn col) -> mlp2 into o-quad [128, 4,
    256] PSUM -> ONE DVE tensor_add (+pixels residual) -> bf16 granule
    store. First 2 pixel granules prefetched during the phase-1 tail on
    the idle gpsimd queue; the rest stream on the scalar HWDGE queue
    (free after W) so the sync queue only carries output stores.
  Output bf16; host upcasts to f32.
"""
import sys
from contextlib import ExitStack

import ml_dtypes
import numpy as np

sys.path.insert(0, "/opt/trn_rl_repo")

import concourse.bass as bass  # noqa: E402
import concourse.tile as tile  # noqa: E402
from concourse import bacc, mybir  # noqa: E402

BF16 = mybir.dt.bfloat16
F32 = mybir.dt.float32
AF = mybir.ActivationFunctionType
MULT = mybir.AluOpType.mult

RMS_EPS = 1.1920928955078125e-07

N_CORES = 8
BS = 2048
NPIX = 256
D = 128
PD = 768  # patch_dim
KC = PD // 128  # 6 contraction chunks
NJ = 2 * D  # 256 j-tiles total (128 per half)
JG = 8  # j-tiles per W mega-slab DMA (1.5MB each)
PIXG = 8  # samples per pixel DMA granule (= 2 quads)


def build_program(B, zero_bias):
    """Build the per-core Bass program for a shard of B samples."""
    assert B % PIXG == 0 and B <= 256
    npair = B // 2
    nquad = B // 4
    ngran = B // PIXG
    nc = bacc.Bacc("TRN2", target_bir_lowering=False, debug=False,
                   num_devices=N_CORES)

    pixG_d = nc.dram_tensor("pixG", (ngran, D, PIXG, NPIX), BF16,
                            kind="ExternalInput")
    patG_d = nc.dram_tensor("patG", (128, KC, B), BF16, kind="ExternalInput")
    w_d = nc.dram_tensor("W", (2, D // JG, 128, KC * JG * 128), BF16,
                         kind="ExternalInput")
    bias_d = nc.dram_tensor("Bias", (2, D, D), F32, kind="ExternalInput")
    nwc_d = nc.dram_tensor("normwc", (D, 1), F32, kind="ExternalInput")
    id_d = nc.dram_tensor("ident", (D, D), BF16, kind="ExternalInput")
    outG_d = nc.dram_tensor("outG", (ngran, D, PIXG, NPIX), BF16,
                            kind="ExternalOutput")

    with tile.TileContext(nc) as tc, ExitStack() as ctx:
        const = ctx.enter_context(tc.tile_pool(name="const", bufs=1))
        bigp = ctx.enter_context(tc.tile_pool(name="big", bufs=1))
        # first phase-2 pixel granule; hoisted so the phase-1 tail can
        # prefetch it on the then-idle gpsimd queue
        pixpre = ctx.enter_context(tc.tile_pool(name="pixpre", bufs=1))

        # constants / persistent tiles. Only patches gate the first
        # matmul; the small constants load on gpsimd (SWDGE) so the
        # HWDGE queues go straight to the first W slab.
        pats = const.tile([128, KC, B], BF16, tag="pat")
        nc.sync.dma_start(pats[:, 0:KC // 2, :],
                          patG_d.ap()[:, 0:KC // 2, :])
        nc.scalar.dma_start(pats[:, KC // 2:, :],
                            patG_d.ap()[:, KC // 2:, :])
        # bias / norm_w / ident are not needed until late phase 1 /
        # phase 2 -- keep the gpsimd SWDGE queue clear for the stats
        # granules (bias alone is 128KB ahead of granule 0 otherwise)
        bt = const.tile([D, 2, D], F32, tag="bias")
        if not zero_bias:
            nc.gpsimd.dma_start(bt[:],
                                bias_d.ap().rearrange("h i j -> i h j"))
        nwc = const.tile([D, 1], F32, tag="normwc")
        ident = const.tile([D, D], BF16, tag="ident")
        maskones = const.tile([128, 2 * D + 1], BF16, tag="maskones")
        nc.vector.memset(maskones[:], 0.0)
        nc.vector.memset(maskones[:, D:D + 1], 1.0)
        epsb = const.tile([128, 1], F32, tag="epsb")
        nc.vector.memset(epsb[:], RMS_EPS)
        inv_cn = const.tile([D, B], F32, tag="invcn")
        invp = const.tile([128, 2 * NPIX], BF16, tag="invp")
        rms_t = const.tile([128, 2 * NPIX], F32, tag="rmst")
        # cn sqrt scratch shares rms_t's space (used before it, at it==64)
        cn_tmp = rms_t[:, 0:B]

        # big laid out [i, s, jj]: per-sample stationaries contiguous
        big = bigp.tile([128, B, NJ], BF16, tag="big")

        pix_cur = {}

        def ph2_preload(g):
            pp = pixpre.tile([128, PIXG, NPIX], BF16, tag="pixpre")
            nc.gpsimd.dma_start(pp[:], pixG_d.ap()[g])
            pix_cur[g] = pp

        # ---- Phase 1: params gen + cn2 + spread pixel RMS stats ----
        with tc.tile_pool(name="wslab", bufs=3) as wpool, \
             tc.tile_pool(name="sq1", bufs=5) as sqpool, \
             tc.tile_pool(name="pixa", bufs=3) as pixap, \
             tc.tile_pool(name="sqg", bufs=3) as sqgp, \
             tc.tile_pool(name="mm1ps", bufs=5, space="PSUM") as mm1ps, \
             tc.tile_pool(name="cn2ps", bufs=1, space="PSUM") as cn2ps, \
             tc.tile_pool(name="msps", bufs=1, space="PSUM") as msps:
            cn2 = cn2ps.tile([D, B], F32, tag="cn2")
            msp = msps.tile([128, 2 * NPIX], F32, tag="msp")
            pending_cn2 = []
            pending_ms = []
            pixa_cur = {}
            sqg_cur = {}

            def emit_cn2(j, sq):
                nc.tensor.matmul(cn2[:], maskones[:, D - j:2 * D - j], sq[:],
                                 start=(j == 0), stop=(j == D - 1))

            def emit_ms(t, mov):
                nc.tensor.matmul(msp[:],
                                 maskones[:, D - t:2 * D - t], mov,
                                 start=(t == 0), stop=(t == npair - 1))

            def stats_stage(t):
                # pair t (samples 2t, 2t+1). Granule-sized DVE square
                # once per 4 pairs; gpsimd SWDGE load 2 granules ahead.
                g = t // 4
                if t % 4 == 0:
                    sq = sqgp.tile([128, PIXG, NPIX], BF16, tag="sqg")
                    pa = pixa_cur.pop(g)
                    nc.vector.tensor_mul(sq[:], pa[:], pa[:])
                    sqg_cur[g] = sq
                    if g + 3 < ngran:
                        pp = pixap.tile([128, PIXG, NPIX], BF16, tag="pixa")
                        nc.gpsimd.dma_start(pp[:], pixG_d.ap()[g + 3])
                        pixa_cur[g + 3] = pp
                    if g - 3 in sqg_cur:
                        del sqg_cur[g - 3]
                u = t % 4
                pending_ms.append((t, sqg_cur[g][:, 2 * u:2 * u + 2, :]))
                if len(pending_ms) > 8:
                    emit_ms(*pending_ms.pop(0))

            # prefetch first three stats granules (gpsimd: HWDGE queues
            # are busy with patches + the first W slab)
            for g in range(3):
                pp = pixap.tile([128, PIXG, NPIX], BF16, tag="pixa")
                nc.gpsimd.dma_start(pp[:], pixG_d.ap()[g])
                pixa_cur[g] = pp

            jt = 0  # stats pair counter
            stats_done = False
            it = 0  # global j-quad iteration (0..63)
            for half in range(2):
                for jg in range(D // JG):
                    sl = wpool.tile([128, KC, JG * 128], BF16, tag="wslab")
                    src_ap = w_d.ap()[half, jg].rearrange(
                        "p (k j) -> p k j", k=KC)
                    if half == 0 and jg == 0:
                        # split first slab per k-chunk across both HWDGE
                        # queues so the k=0 matmuls start ~5us in
                        for k in range(KC):
                            dma_eng = nc.sync if k % 2 == 0 else nc.scalar
                            dma_eng.dma_start(sl[:, k, :], src_ap[:, k, :])
                    elif half == 0 and jg <= 3:
                        # ramp: split early slabs too so neither queue
                        # falls behind the MM consumption rate
                        nc.sync.dma_start(sl[:, 0:KC // 2, :],
                                          src_ap[:, 0:KC // 2, :])
                        nc.scalar.dma_start(sl[:, KC // 2:, :],
                                            src_ap[:, KC // 2:, :])
                    else:
                        dma_eng = nc.sync if jg % 2 == 0 else nc.scalar
                        dma_eng.dma_start(sl[:], src_ap)
                    for jp in range(JG // 2):  # j-tile pairs
                        j0 = jg * JG + 2 * jp
                        jj0 = half * D + j0
                        ps = mm1ps.tile([D, 2, B], F32, tag="mm1")
                        for u in range(2):
                            for k in range(KC):
                                nc.tensor.matmul(
                                    ps[:, u, :],
                                    sl[:, k, (2 * jp + u) * 128:
                                       (2 * jp + u + 1) * 128],
                                    pats[:, k, :], start=(k == 0),
                                    stop=(k == KC - 1))
                        # evac to big[:, :, jj0:jj0+2]: dst in natural
                        # order (j-pairs contiguous -> full-word bf16
                        # writes, no RMW); source PSUM AP transposed
                        if zero_bias:
                            nc.scalar.activation(
                                big[:, :, jj0:jj0 + 2],
                                ps[:].rearrange("p u s -> p s u"),
                                AF.Identity)
                        else:
                            for u in range(2):
                                nc.scalar.activation(
                                    big[:, :, jj0 + u], ps[:, u, :],
                                    AF.Identity,
                                    bias=bt[:, half, j0 + u:j0 + u + 1])
                        if half == 0:
                            # square on ACT (one PSUM read; Square is in
                            # every act table). With nonzero bias the
                            # square must see post-bias values (big).
                            sq = sqpool.tile([D, 2, B], BF16, tag="sq")
                            if zero_bias:
                                nc.scalar.activation(sq[:], ps[:],
                                                     AF.Square)
                            else:
                                for u in range(2):
                                    nc.vector.tensor_mul(
                                        sq[:, u, :], big[:, :, jj0 + u],
                                        big[:, :, jj0 + u])
                            pending_cn2.append((j0, sq[:, 0, :]))
                            pending_cn2.append((j0 + 1, sq[:, 1, :]))
                            while len(pending_cn2) > 4:
                                emit_cn2(*pending_cn2.pop(0))
                        # spread stats: ~1.1 pairs per iteration starting
                        # at iter 4 (granule 0 needs ~20us on SWDGE);
                        # done by iter ~119 of 128
                        target = (0 if it < 4 else
                                  min(npair, (it - 3) + (it - 3) // 9))
                        while jt < target:
                            stats_stage(jt)
                            jt += 1
                        if jt == npair and not stats_done:
                            stats_done = True
                            for args in pending_ms:
                                emit_ms(*args)
                            pending_ms = []
                            nc.scalar.activation(rms_t[:], msp[:], AF.Sqrt,
                                                 bias=epsb[:],
                                                 scale=1.0 / D)
                            with nc.allow_low_precision("bf16 inv-rms"):
                                nc.vector.reciprocal(invp[:], rms_t[:])
                        it += 1
                        if it == 64:
                            # half-0 done: flush cn2, compute inv_cn so
                            # the sqrt/table switches hide under half-1
                            for args in pending_cn2:
                                emit_cn2(*args)
                            pending_cn2 = []
                            nc.scalar.activation(cn_tmp[:], cn2[:],
                                                 AF.Sqrt)
                            nc.vector.tensor_scalar_max(cn_tmp[:],
                                                        cn_tmp[:], 1e-12)
                            nc.vector.reciprocal(inv_cn[:], cn_tmp[:])
                        if it == 104:
                            nc.gpsimd.dma_start(nwc[:], nwc_d.ap())
                            nc.gpsimd.dma_start(ident[:], id_d.ap())
                        if it == 112:
                            ph2_preload(0)
            assert not pending_cn2 and not pending_ms and stats_done

        # ---- Phase 2: per-quad MLP pipeline ----
        with tc.tile_pool(name="pix2", bufs=6) as pixp, \
             tc.tile_pool(name="xn", bufs=6) as xnp, \
             tc.tile_pool(name="sh", bufs=5) as shp, \
             tc.tile_pool(name="ot", bufs=4) as otp, \
             tc.tile_pool(name="sbcps", bufs=2, space="PSUM") as sbcps, \
             tc.tile_pool(name="hps", bufs=2, space="PSUM") as hpsp, \
             tc.tile_pool(name="ops", bufs=2, space="PSUM") as opsp:

            stA = {}
            stB = {}
            stC = {}
            ot_cur = {}

            def ph2_load(g):
                pp = pixp.tile([128, PIXG, NPIX], BF16, tag="pix2")
                nc.scalar.dma_start(pp[:], pixG_d.ap()[g])
                pix_cur[g] = pp

            def quad_pix(q):  # [128, 4, 256] slice of the granule
                pp = pix_cur[q // 2]
                qi = q % 2
                return pp[:, 4 * qi:4 * qi + 4, :]

            def pair_pix(t):  # [128, 2, 256] slice of the granule
                pp = pix_cur[t // 4]
                ti = t % 4
                return pp[:, 2 * ti:2 * ti + 2, :]

            def stage_a(t):  # inv-rms broadcast MM for pair t
                sbc = sbcps.tile([D, 2 * NPIX], F32, tag="sbc")
                nc.tensor.matmul(
                    sbc[:],
                    ident[:, t:t + 1].to_broadcast((D, D)), invp[:])
                stA[t] = sbc

            def stage_b(t):  # xn = pp * nw * inv, one pair-wide DVE stt
                sbc = stA.pop(t)
                xn = xnp.tile([D, 2, NPIX], BF16, tag="xn")
                nc.vector.scalar_tensor_tensor(
                    xn[:], pair_pix(t), nwc[:],
                    sbc[:].rearrange("p (u n) -> p u n", u=2),
                    op0=MULT, op1=MULT)
                stB[t] = xn

            def stage_c(q):  # mlp1 + silu per sample
                xns = [stB.pop(2 * q), stB.pop(2 * q + 1)]
                shs = shp.tile([D, 4, NPIX], BF16, tag="sh")
                hs = []
                for v in range(2):  # pair within quad
                    h = hpsp.tile([D, 2, NPIX], F32, tag="h")
                    for u in range(2):
                        s = 4 * q + 2 * v + u
                        nc.tensor.matmul(h[:, u, :], big[:, s, 0:D],
                                         xns[v][:, u, :])
                    hs.append(h)
                for v in range(2):
                    for u in range(2):
                        s = 4 * q + 2 * v + u
                        nc.scalar.activation(shs[:, 2 * v + u, :],
                                             hs[v][:, u, :], AF.Silu,
                                             scale=inv_cn[:, s:s + 1])
                stC[q] = shs

            def stage_d(q):  # mlp2 + residual + evac, quad granularity
                shs = stC.pop(q)
                if q % 2 == 0:
                    ot = otp.tile([128, PIXG, NPIX], BF16, tag="ot",
                                  name="ot")
                    ot_cur[q // 2] = ot
                ot = ot_cur[q // 2]
                qi = q % 2
                o = opsp.tile([D, 4, NPIX], F32, tag="o")
                for v in range(4):
                    s = 4 * q + v
                    nc.tensor.matmul(o[:, v, :], big[:, s, D:2 * D],
                                     shs[:, v, :])
                nc.vector.tensor_add(
                    ot[:, 4 * qi:4 * qi + 4, :], o[:], quad_pix(q))
                if q // 2 == ngran - 1:
                    # last granule: store per quad so the tail drains
                    # as soon as each residual lands
                    nc.sync.dma_start(
                        outG_d.ap()[q // 2][:, 4 * qi:4 * qi + 4, :],
                        ot[:, 4 * qi:4 * qi + 4, :])
                    if qi == 1:
                        del pix_cur[q // 2]
                elif qi == 1:
                    nc.sync.dma_start(outG_d.ap()[q // 2], ot[:])
                    del pix_cur[q // 2]

            for q in range(nquad + 3):
                # stt (stage_b) is emitted FIRST so the DVE FIFO runs
                # the stts -- whose inputs are ready -- while the PE
                # produces o(q-3); the residual tensor_add then never
                # blocks the DVE at the head of the queue. mlp2 before
                # mlp1 on the PE so o(q-3) is ready as soon as possible.
                # sbc tiles are per-PAIR (1 PSUM bank) so a 2-buf pool
                # plus a 2-buf o-quad pool fits the 8 banks.
                if 1 <= q < nquad + 1:
                    stage_b(2 * (q - 1))
                    stage_b(2 * (q - 1) + 1)
                if q >= 3:
                    stage_d(q - 3)
                if 2 <= q < nquad + 2:
                    stage_c(q - 2)
                if q == 0:
                    for g in range(1, min(5, ngran)):
                        ph2_load(g)
                elif q % 2 == 0 and 5 <= q // 2 + 4 < ngran:
                    ph2_load(q // 2 + 4)
                if q < nquad:
                    stage_a(2 * q)
                    stage_a(2 * q + 1)

    nc.compile()
    return nc


def host_prep(pixels, patches, W_pg, b_pg, norm_w):
    bf = ml_dtypes.bfloat16
    # pixels (BS, NPIX, D) -> granule-major (BS//PIXG, D, PIXG, NPIX):
    # 4KB contiguous per partition per granule
    pixG = np.ascontiguousarray(
        pixels.reshape(BS // PIXG, PIXG, NPIX, D).transpose(0, 3, 1, 2)
        .astype(bf))
    # patches (BS, PD) -> (128, KC, BS)
    patG = np.ascontiguousarray(
        patches.T.reshape(KC, 128, BS).transpose(1, 0, 2).astype(bf))
    # W_pg (2*D*D, PD): layer[i, j] = W_pg[half*D*D + i*128 + j, :]
    # device j-tile stationary needs [k-part(128), j-in-tile -> i rows]
    # slab-major: (2, D//JG, 128(p), KC, JG, 128(i)) flattened last 3
    Wp = W_pg.reshape(2, D, D, PD).transpose(0, 3, 2, 1)   # (2, PD, j, i)
    Wp = Wp.reshape(2, KC, 128, D // JG, JG, D)            # (2,k,p,jg,jl,i)
    Wp = np.ascontiguousarray(Wp.transpose(0, 3, 2, 1, 4, 5))  # 2,jg,p,k,jl,i
    Wh = Wp.reshape(2, D // JG, 128, KC * JG * 128).astype(bf)
    Bias = np.ascontiguousarray(b_pg.reshape(2, D, D)).astype(np.float32)
    nwc = np.ascontiguousarray(norm_w.reshape(D, 1)).astype(np.float32)
    ident = np.eye(D, dtype=bf)
    return pixG, patG, Wh, Bias, nwc, ident


_NC_CACHE = {}


def _run(pixels, patches, W_pg, b_pg, norm_w, **spmd_kwargs):
    from concourse.bass_utils import run_bass_kernel_spmd

    pixG, patG, Wh, Bias, nwc, ident = host_prep(
        pixels, patches, W_pg, b_pg, norm_w)
    B = pixels.shape[0] // N_CORES
    zero_bias = not np.any(b_pg)
    key = (B, zero_bias)
    if key not in _NC_CACHE:
        _NC_CACHE[key] = build_program(B, zero_bias)
    nc = _NC_CACHE[key]

    gpc = B // PIXG  # granules per core
    in_maps = []
    for c in range(N_CORES):
        in_maps.append({
            "pixG": pixG[c * gpc:(c + 1) * gpc],
            "patG": np.ascontiguousarray(patG[:, :, c * B:(c + 1) * B]),
            "W": Wh,
            "Bias": Bias,
            "normwc": nwc,
            "ident": ident,
        })
    try:
        res = run_bass_kernel_spmd(nc, in_maps, list(range(N_CORES)),
                                   **spmd_kwargs)
    except Exception:
        # transient device wedge (NRT_EXEC_UNIT_UNRECOVERABLE) — retry once
        res = run_bass_kernel_spmd(nc, in_maps, list(range(N_CORES)),
                                   **spmd_kwargs)
    outG = np.concatenate([res.results[c]["outG"] for c in range(N_CORES)], 0)
    # (BS//PIXG, D, PIXG, NPIX) -> (BS, NPIX, D) f32
    out = np.ascontiguousarray(
        outG.astype(np.float32).transpose(0, 2, 3, 1).reshape(BS, NPIX, D))
    return out, res


def kernel(pixels, patches, W_pg, b_pg, norm_w):
    out, _ = _run(pixels, patches, W_pg, b_pg, norm_w)
    return out


if __name__ == "__main__":
    rng = np.random.default_rng(0)
    inputs = {
        "pixels": rng.standard_normal((BS, NPIX, D), dtype=np.float32),
        "patches": rng.standard_normal((BS, PD), dtype=np.float32),
        "W_pg": (rng.standard_normal((2 * D * D, PD)) * 0.02).astype(np.float32),
        "b_pg": np.zeros((2 * D * D,), np.float32),
        "norm_w": np.ones((D,), np.float32),
    }
    out = kernel(**inputs)
    print(out.shape, out.dtype)


# revision 42
# speedup vs baseline: 1.0302x; 1.0302x over previous
"""NerfHead Trainium2 kernel: per-sample generated 2-layer MLP over pixels.

Sharding: pure data parallel over the batch dim across 8 cores.
Host does all layout permutations / dtype casts (restaging only).

Per core (B=256 samples):
  Phase 1: params j-tiles [128, B] PSUM (pair-batched ACT evac when bias
    is all-zero) into big [128(i), B, NJ] so per-sample mlp stationaries
    are CONTIGUOUS (fast LDWEIGHTS). W streamed as 1.5MB slab-major
    host-prepped DMAs, alternating sync/scalar HWDGE queues; the first
    slab is split per k-chunk across both queues so matmuls start ~5us
    in (earlier HAM warm-up). cn2 via squares + masked-ones MM
    accumulation. Pixel RMS stats spread UNIFORMLY over the whole phase
    (~1.06 pairs/iter) on gpsimd SWDGE granule loads + granule-sized
    DVE squares, so the stats DMA never oversubscribes the W stream and
    the masked-ones ms matmuls never stall the PE queue.
  Normalizers: batched ACT Sqrt + DVE reciprocal.
  Phase 2 (per quad = 4 samples): 2 sbc broadcast MMs (ident-col,
    N=512) into a 2-bank PSUM quad -> ONE DVE stt xn = pp*nw*inv ->
    mlp1 -> ACT Silu(scale=inv_cn col) -> mlp2 into o-quad [128, 4,
    256] PSUM -> ONE DVE tensor_add (+pixels residual) -> bf16 granule
    store. First 2 pixel granules prefetched during the phase-1 tail on
    the idle gpsimd queue; the rest stream on the scalar HWDGE queue
    (free after W) so the sync queue only carries output stores.
  Output bf16; host upcasts to f32.
"""
import sys
from contextlib import ExitStack

import ml_dtypes
import numpy as np

sys.path.insert(0, "/opt/trn_rl_repo")

import concourse.bass as bass  # noqa: E402
import concourse.tile as tile  # noqa: E402
from concourse import bacc, mybir  # noqa: E402

BF16 = mybir.dt.bfloat16
F32 = mybir.dt.float32
AF = mybir.ActivationFunctionType
MULT = mybir.AluOpType.mult

RMS_EPS = 1.1920928955078125e-07

N_CORES = 8
BS = 2048
NPIX = 256
D = 128
PD = 768  # patch_dim
KC = PD // 128  # 6 contraction chunks
NJ = 2 * D  # 256 j-tiles total (128 per half)
JG = 8  # j-tiles per W mega-slab DMA (1.5MB each)
PIXG = 8  # samples per pixel DMA granule (= 2 quads)


def build_program(B, zero_bias):
    """Build the per-core Bass program for a shard of B samples."""
    assert B % PIXG == 0 and B <= 256
    npair = B // 2
    nquad = B // 4
    ngran = B // PIXG
    nc = bacc.Bacc("TRN2", target_bir_lowering=False, debug=False,
                   num_devices=N_CORES)

    pixG_d = nc.dram_tensor("pixG", (ngran, D, PIXG, NPIX), BF16,
                            kind="ExternalInput")
    patG_d = nc.dram_tensor("patG", (128, KC, B), BF16, kind="ExternalInput")
    w_d = nc.dram_tensor("W", (2, D // JG, 128, KC * JG * 128), BF16,
                         kind="ExternalInput")
    bias_d = nc.dram_tensor("Bias", (2, D, D), F32, kind="ExternalInput")
    nwc_d = nc.dram_tensor("normwc", (D, 1), F32, kind="ExternalInput")
    id_d = nc.dram_tensor("ident", (D, D), BF16, kind="ExternalInput")
    outG_d = nc.dram_tensor("outG", (ngran, D, PIXG, NPIX), BF16,
                            kind="ExternalOutput")

    with tile.TileContext(nc) as tc, ExitStack() as ctx:
        const = ctx.enter_context(tc.tile_pool(name="const", bufs=1))
        bigp = ctx.enter_context(tc.tile_pool(name="big", bufs=1))
        # first phase-2 pixel granule; hoisted so the phase-1 tail can
        # prefetch it on the then-idle gpsimd queue
        pixpre = ctx.enter_context(tc.tile_pool(name="pixpre", bufs=1))

        # constants / persistent tiles. Only patches gate the first
        # matmul; the small constants load on gpsimd (SWDGE) so the
        # HWDGE queues go straight to the first W slab.
        pats = const.tile([128, KC, B], BF16, tag="pat")
        nc.sync.dma_start(pats[:, 0:KC // 2, :],
                          patG_d.ap()[:, 0:KC // 2, :])
        nc.scalar.dma_start(pats[:, KC // 2:, :],
                            patG_d.ap()[:, KC // 2:, :])
        # bias / norm_w / ident are not needed until late phase 1 /
        # phase 2 -- keep the gpsimd SWDGE queue clear for the stats
        # granules (bias alone is 128KB ahead of granule 0 otherwise)
        bt = const.tile([D, 2, D], F32, tag="bias")
        if not zero_bias:
            nc.gpsimd.dma_start(bt[:],
                                bias_d.ap().rearrange("h i j -> i h j"))
        nwc = const.tile([D, 1], F32, tag="normwc")
        ident = const.tile([D, D], BF16, tag="ident")
        maskones = const.tile([128, 2 * D + 1], BF16, tag="maskones")
        nc.vector.memset(maskones[:], 0.0)
        nc.vector.memset(maskones[:, D:D + 1], 1.0)
        epsb = const.tile([128, 1], F32, tag="epsb")
        nc.vector.memset(epsb[:], RMS_EPS)
        inv_cn = const.tile([D, B], F32, tag="invcn")
        invp = const.tile([128, 2 * NPIX], BF16, tag="invp")
        rms_t = const.tile([128, 2 * NPIX], F32, tag="rmst")
        # cn sqrt scratch shares rms_t's space (used before it, at it==64)
        cn_tmp = rms_t[:, 0:B]

        # big laid out [i, s, jj]: per-sample stationaries contiguous
        big = bigp.tile([128, B, NJ], BF16, tag="big")

        pix_cur = {}

        def ph2_preload(g):
            pp = pixpre.tile([128, PIXG, NPIX], BF16, tag="pixpre")
            nc.gpsimd.dma_start(pp[:], pixG_d.ap()[g])
            pix_cur[g] = pp

        # ---- Phase 1: params gen + cn2 + spread pixel RMS stats ----
        with tc.tile_pool(name="wslab", bufs=3) as wpool, \
             tc.tile_pool(name="sq1", bufs=5) as sqpool, \
             tc.tile_pool(name="pixa", bufs=3) as pixap, \
             tc.tile_pool(name="sqg", bufs=3) as sqgp, \
             tc.tile_pool(name="mm1ps", bufs=5, space="PSUM") as mm1ps, \
             tc.tile_pool(name="cn2ps", bufs=1, space="PSUM") as cn2ps, \
             tc.tile_pool(name="msps", bufs=1, space="PSUM") as msps:
            cn2 = cn2ps.tile([D, B], F32, tag="cn2")
            msp = msps.tile([128, 2 * NPIX], F32, tag="msp")
            pending_cn2 = []
            pending_ms = []
            pixa_cur = {}
            sqg_cur = {}

            def emit_cn2(j, sq):
                nc.tensor.matmul(cn2[:], maskones[:, D - j:2 * D - j], sq[:],
                                 start=(j == 0), stop=(j == D - 1))

            def emit_ms(t, mov):
                nc.tensor.matmul(msp[:],
                                 maskones[:, D - t:2 * D - t], mov,
                                 start=(t == 0), stop=(t == npair - 1))

            def stats_stage(t):
                # pair t (samples 2t, 2t+1). Granule-sized DVE square
                # once per 4 pairs; gpsimd SWDGE load 2 granules ahead.
                g = t // 4
                if t % 4 == 0:
                    sq = sqgp.tile([128, PIXG, NPIX], BF16, tag="sqg")
                    pa = pixa_cur.pop(g)
                    nc.vector.tensor_mul(sq[:], pa[:], pa[:])
                    sqg_cur[g] = sq
                    if g + 3 < ngran:
                        pp = pixap.tile([128, PIXG, NPIX], BF16, tag="pixa")
                        nc.gpsimd.dma_start(pp[:], pixG_d.ap()[g + 3])
                        pixa_cur[g + 3] = pp
                    if g - 3 in sqg_cur:
                        del sqg_cur[g - 3]
                u = t % 4
                pending_ms.append((t, sqg_cur[g][:, 2 * u:2 * u + 2, :]))
                if len(pending_ms) > 8:
                    emit_ms(*pending_ms.pop(0))

            # prefetch first three stats granules (gpsimd: HWDGE queues
            # are busy with patches + the first W slab)
            for g in range(3):
                pp = pixap.tile([128, PIXG, NPIX], BF16, tag="pixa")
                nc.gpsimd.dma_start(pp[:], pixG_d.ap()[g])
                pixa_cur[g] = pp

            jt = 0  # stats pair counter
            stats_done = False
            it = 0  # global j-quad iteration (0..63)
            for half in range(2):
                for jg in range(D // JG):
                    sl = wpool.tile([128, KC, JG * 128], BF16, tag="wslab")
                    src_ap = w_d.ap()[half, jg].rearrange(
                        "p (k j) -> p k j", k=KC)
                    if half == 0 and jg == 0:
                        # split first slab per k-chunk across both HWDGE
                        # queues so the k=0 matmuls start ~5us in
                        for k in range(KC):
                            dma_eng = nc.sync if k % 2 == 0 else nc.scalar
                            dma_eng.dma_start(sl[:, k, :], src_ap[:, k, :])
                    elif half == 0 and jg <= 3:
                        # ramp: split early slabs too so neither queue
                        # falls behind the MM consumption rate
                        nc.sync.dma_start(sl[:, 0:KC // 2, :],
                                          src_ap[:, 0:KC // 2, :])
                        nc.scalar.dma_start(sl[:, KC // 2:, :],
                                            src_ap[:, KC // 2:, :])
                    else:
                        dma_eng = nc.sync if jg % 2 == 0 else nc.scalar
                        dma_eng.dma_start(sl[:], src_ap)
                    for jp in range(JG // 2):  # j-tile pairs
                        j0 = jg * JG + 2 * jp
                        jj0 = half * D + j0
                        ps = mm1ps.tile([D, 2, B], F32, tag="mm1")
                        for u in range(2):
                            for k in range(KC):
                                nc.tensor.matmul(
                                    ps[:, u, :],
                                    sl[:, k, (2 * jp + u) * 128:
                                       (2 * jp + u + 1) * 128],
                                    pats[:, k, :], start=(k == 0),
                                    stop=(k == KC - 1))
                        # evac to big[:, :, jj0:jj0+2]: dst in natural
                        # order (j-pairs contiguous -> full-word bf16
                        # writes, no RMW); source PSUM AP transposed
                        if zero_bias:
                            nc.scalar.activation(
                                big[:, :, jj0:jj0 + 2],
                                ps[:].rearrange("p u s -> p s u"),
                                AF.Identity)
                        else:
                            for u in range(2):
                                nc.scalar.activation(
                                    big[:, :, jj0 + u], ps[:, u, :],
                                    AF.Identity,
                                    bias=bt[:, half, j0 + u:j0 + u + 1])
                        if half == 0:
                            # square on ACT (one PSUM read; Square is in
                            # every act table). With nonzero bias the
                            # square must see post-bias values (big).
                            sq = sqpool.tile([D, 2, B], BF16, tag="sq")
                            if zero_bias:
                                nc.scalar.activation(sq[:], ps[:],
                                                     AF.Square)
                            else:
                                for u in range(2):
                                    nc.vector.tensor_mul(
                                        sq[:, u, :], big[:, :, jj0 + u],
                                        big[:, :, jj0 + u])
                            pending_cn2.append((j0, sq[:, 0, :]))
                            pending_cn2.append((j0 + 1, sq[:, 1, :]))
                            while len(pending_cn2) > 4:
                                emit_cn2(*pending_cn2.pop(0))
                        # spread stats: ~1.1 pairs per iteration starting
                        # at iter 4 (granule 0 needs ~20us on SWDGE);
                        # done by iter ~119 of 128
                        target = (0 if it < 4 else
                                  min(npair, (it - 3) + (it - 3) // 9))
                        while jt < target:
                            stats_stage(jt)
                            jt += 1
                        if jt == npair and not stats_done:
                            stats_done = True
                            for args in pending_ms:
                                emit_ms(*args)
                            pending_ms = []
                            nc.scalar.activation(rms_t[:], msp[:], AF.Sqrt,
                                                 bias=epsb[:],
                                                 scale=1.0 / D)
                            with nc.allow_low_precision("bf16 inv-rms"):
                                nc.vector.reciprocal(invp[:], rms_t[:])
                        it += 1
                        if it == 64:
                            # half-0 done: flush cn2, compute inv_cn so
                            # the sqrt/table switches hide under half-1
                            for args in pending_cn2:
                                emit_cn2(*args)
                            pending_cn2 = []
                            nc.scalar.activation(cn_tmp[:], cn2[:],
                                                 AF.Sqrt)
                            nc.vector.tensor_scalar_max(cn_tmp[:],
                                                        cn_tmp[:], 1e-12)
                            nc.vector.reciprocal(inv_cn[:], cn_tmp[:])
                        if it == 104:
                            nc.gpsimd.dma_start(nwc[:], nwc_d.ap())
                            nc.gpsimd.dma_start(ident[:], id_d.ap())
                        if it == 112:
                            ph2_preload(0)
            assert not pending_cn2 and not pending_ms and stats_done

        # ---- Phase 2: per-quad MLP pipeline ----
        with tc.tile_pool(name="pix2", bufs=6) as pixp, \
             tc.tile_pool(name="xn", bufs=4) as xnp, \
             tc.tile_pool(name="sh", bufs=5) as shp, \
             tc.tile_pool(name="ot", bufs=4) as otp, \
             tc.tile_pool(name="sbcps", bufs=1, space="PSUM") as sbcps, \
             tc.tile_pool(name="hps", bufs=2, space="PSUM") as hpsp, \
             tc.tile_pool(name="ops", bufs=2, space="PSUM") as opsp:

            stA = {}
            stB = {}
            stC = {}
            ot_cur = {}

            def ph2_load(g):
                pp = pixp.tile([128, PIXG, NPIX], BF16, tag="pix2")
                nc.scalar.dma_start(pp[:], pixG_d.ap()[g])
                pix_cur[g] = pp

            def quad_pix(q):  # [128, 4, 256] slice of the granule
                pp = pix_cur[q // 2]
                qi = q % 2
                return pp[:, 4 * qi:4 * qi + 4, :]

            def pair_pix(t):  # [128, 2, 256] slice of the granule
                pp = pix_cur[t // 4]
                ti = t % 4
                return pp[:, 2 * ti:2 * ti + 2, :]

            def stage_a(q):  # inv-rms broadcast MMs for quad q
                sbc = sbcps.tile([D, 2, 2 * NPIX], F32, tag="sbc")
                for i in range(2):
                    t = 2 * q + i
                    nc.tensor.matmul(
                        sbc[:, i, :],
                        ident[:, t:t + 1].to_broadcast((D, D)), invp[:])
                stA[q] = sbc

            def stage_b(q):  # xn = pp * nw * inv, one quad-wide DVE stt
                sbc = stA.pop(q)
                xn = xnp.tile([D, 4, NPIX], BF16, tag="xn")
                nc.vector.scalar_tensor_tensor(
                    xn[:], quad_pix(q), nwc[:],
                    sbc[:].rearrange("p i (u n) -> p (i u) n", u=2),
                    op0=MULT, op1=MULT)
                stB[q] = xn

            def stage_c(q):  # mlp1 + silu per sample
                xn = stB.pop(q)
                shs = shp.tile([D, 4, NPIX], BF16, tag="sh")
                hs = []
                for v in range(2):  # pair within quad
                    h = hpsp.tile([D, 2, NPIX], F32, tag="h")
                    for u in range(2):
                        s = 4 * q + 2 * v + u
                        nc.tensor.matmul(h[:, u, :], big[:, s, 0:D],
                                         xn[:, 2 * v + u, :])
                    hs.append(h)
                for v in range(2):
                    for u in range(2):
                        s = 4 * q + 2 * v + u
                        nc.scalar.activation(shs[:, 2 * v + u, :],
                                             hs[v][:, u, :], AF.Silu,
                                             scale=inv_cn[:, s:s + 1])
                stC[q] = shs

            def stage_d(q):  # mlp2 + residual + evac, quad granularity
                shs = stC.pop(q)
                if q % 2 == 0:
                    ot = otp.tile([128, PIXG, NPIX], BF16, tag="ot",
                                  name="ot")
                    ot_cur[q // 2] = ot
                ot = ot_cur[q // 2]
                qi = q % 2
                o = opsp.tile([D, 4, NPIX], F32, tag="o")
                for v in range(4):
                    s = 4 * q + v
                    nc.tensor.matmul(o[:, v, :], big[:, s, D:2 * D],
                                     shs[:, v, :])
                nc.vector.tensor_add(
                    ot[:, 4 * qi:4 * qi + 4, :], o[:], quad_pix(q))
                if q // 2 == ngran - 1:
                    # last granule: store per quad so the tail drains
                    # as soon as each residual lands
                    nc.sync.dma_start(
                        outG_d.ap()[q // 2][:, 4 * qi:4 * qi + 4, :],
                        ot[:, 4 * qi:4 * qi + 4, :])
                    if qi == 1:
                        del pix_cur[q // 2]
                elif qi == 1:
                    nc.sync.dma_start(outG_d.ap()[q // 2], ot[:])
                    del pix_cur[q // 2]

            for q in range(nquad + 3):
                # sbc MMs and their stt are emitted in the SAME
                # iteration: the stt drains the single sbc PSUM buffer
                # immediately (first op in the DVE FIFO each iteration),
                # so a 1-buf sbc pool never stalls and the o-quad pool
                # gets 2 bufs -- the residual tensor_add then never
                # waits on mlp2 PSUM recycling. mlp2 before mlp1 on the
                # PE so o(q-3) is ready as soon as possible.
                if q < nquad:
                    stage_a(q)
                    stage_b(q)
                if q >= 3:
                    stage_d(q - 3)
                if 2 <= q < nquad + 2:
                    stage_c(q - 2)
                if q == 0:
                    for g in range(1, min(5, ngran)):
                        ph2_load(g)
                elif q % 2 == 0 and 5 <= q // 2 + 4 < ngran:
                    ph2_load(q // 2 + 4)

    nc.compile()
    return nc


def host_prep(pixels, patches, W_pg, b_pg, norm_w):
    bf = ml_dtypes.bfloat16
    # pixels (BS, NPIX, D) -> granule-major (BS//PIXG, D, PIXG, NPIX):
    # 4KB contiguous per partition per granule
    pixG = np.ascontiguousarray(
        pixels.reshape(BS // PIXG, PIXG, NPIX, D).transpose(0, 3, 1, 2)
        .astype(bf))
    # patches (BS, PD) -> (128, KC, BS)
    patG = np.ascontiguousarray(
        patches.T.reshape(KC, 128, BS).transpose(1, 0, 2).astype(bf))
    # W_pg (2*D*D, PD): layer[i, j] = W_pg[half*D*D + i*128 + j, :]
    # device j-tile stationary needs [k-part(128), j-in-tile -> i rows]
    # slab-major: (2, D//JG, 128(p), KC, JG, 128(i)) flattened last 3
    Wp = W_pg.reshape(2, D, D, PD).transpose(0, 3, 2, 1)   # (2, PD, j, i)
    Wp = Wp.reshape(2, KC, 128, D // JG, JG, D)            # (2,k,p,jg,jl,i)
    Wp = np.ascontiguousarray(Wp.transpose(0, 3, 2, 1, 4, 5))  # 2,jg,p,k,jl,i
    Wh = Wp.reshape(2, D // JG, 128, KC * JG * 128).astype(bf)
    Bias = np.ascontiguousarray(b_pg.reshape(2, D, D)).astype(np.float32)
    nwc = np.ascontiguousarray(norm_w.reshape(D, 1)).astype(np.float32)
    ident = np.eye(D, dtype=bf)
    return pixG, patG, Wh, Bias, nwc, ident


_NC_CACHE = {}


def _run(pixels, patches, W_pg, b_pg, norm_w, **spmd_kwargs):
    from concourse.bass_utils import run_bass_kernel_spmd

    pixG, patG, Wh, Bias, nwc, ident = host_prep(
        pixels, patches, W_pg, b_pg, norm_w)
    B = pixels.shape[0] // N_CORES
    zero_bias = not np.any(b_pg)
    key = (B, zero_bias)
    if key not in _NC_CACHE:
        _NC_CACHE[key] = build_program(B, zero_bias)
    nc = _NC_CACHE[key]

    gpc = B // PIXG  # granules per core
    in_maps = []
    for c in range(N_CORES):
        in_maps.append({
            "pixG": pixG[c * gpc:(c + 1) * gpc],
            "patG": np.ascontiguousarray(patG[:, :, c * B:(c + 1) * B]),
            "W": Wh,
            "Bias": Bias,
            "normwc": nwc,
            "ident": ident,
        })
    try:
        res = run_bass_kernel_spmd(nc, in_maps, list(range(N_CORES)),
                                   **spmd_kwargs)
    except Exception:
        # transient device wedge (NRT_EXEC_UNIT_UNRECOVERABLE) — retry once
        res = run_bass_kernel_spmd(nc, in_maps, list(range(N_CORES)),
                                   **spmd_kwargs)
    outG = np.concatenate([res.results[c]["outG"] for c in range(N_CORES)], 0)
    # (BS//PIXG, D, PIXG, NPIX) -> (BS, NPIX, D) f32
    out = np.ascontiguousarray(
        outG.astype(np.float32).transpose(0, 2, 3, 1).reshape(BS, NPIX, D))
    return out, res


def kernel(pixels, patches, W_pg, b_pg, norm_w):
    out, _ = _run(pixels, patches, W_pg, b_pg, norm_w)
    return out


if __name__ == "__main__":
    rng = np.random.default_rng(0)
    inputs = {
        "pixels": rng.standard_normal((BS, NPIX, D), dtype=np.float32),
        "patches": rng.standard_normal((BS, PD), dtype=np.float32),
        "W_pg": (rng.standard_normal((2 * D * D, PD)) * 0.02).astype(np.float32),
        "b_pg": np.zeros((2 * D * D,), np.float32),
        "norm_w": np.ones((D,), np.float32),
    }
    out = kernel(**inputs)
    print(out.shape, out.dtype)
